# revision 20
# baseline (speedup 1.0000x reference)
"""Trainium2 Bass kernel for nn_CustomLoss_60885456388844.

Masked-distance custom loss over logits [65536, 1024]:
  probs = exp(logits) / (sum_exp + eps)            per row
  pred  = argmax(logits)                           per row
  same_event = event_ids[pred] == event_ids[gt]
  term1 (same_event): |pred-gt| * sum_{gt range} probs / (range_len + eps)
  term2 (else):       ub * sum_{outside gt range} (1+probs) / (V-range_len + eps)
  loss = sum over rows

Sharding: pure data-parallel on the row axis across 8 NeuronCores
(8192 rows each). Each core reduces its rows to a [128] partial vector;
the host sums the 8x128 partials into the scalar loss.

Per-core device plan (64 tiles of [128 rows x 1024 vocab]):
  DMA   : logits tile -> SBUF
  ScalarE: exp(tile), accum_out -> per-row sum S (free)
  VectorE: 16 per-row block sums of exp (one tensor_reduce)
           max + max_index -> per-row argmax
  Epilogue (batched [128, 64] ops): one-hot block select for the
  gt-range exp sum, then the closed-form per-row loss and a final
  free-axis reduction to [128, 1].
"""

import numpy as np

N = 65536
V = 1024
NCORES = 8
NPC = N // NCORES          # rows per core
P = 128                    # SBUF partitions
TILES = NPC // P           # row tiles per core
NBLK = 16                  # token-range blocks per row
BLK = V // NBLK            # tokens per block
EPS = 1e-10


def _np_loss(logits, gt, event_ids, range_start, range_end):
    """Exact-semantics numpy fallback (only used if the vocab tables do not
    have the contiguous 64-token block structure this kernel hardcodes)."""
    lg = logits.astype(np.float64)
    exp = np.exp(lg)
    sum_exp = exp.sum(axis=1, keepdims=True) + EPS
    probs = exp / sum_exp
    pred = lg.argmax(axis=1)
    ub = float(np.max(range_end - range_start))
    same = event_ids[pred] == event_ids[gt]
    rs = range_start[gt][:, None]
    re_ = range_end[gt][:, None]
    col = np.arange(V)[None, :]
    in_range = (col >= rs) & (col < re_)
    mask1 = (same[:, None] & in_range).astype(np.float64)
    mask2 = np.where(same[:, None], 0.0, np.where(in_range, 0.0, 1.0))
    tok_dist = np.abs(pred - gt).astype(np.float64)[:, None]
    d = (tok_dist * probs * mask1 / (mask1.sum(1, keepdims=True) + EPS)
         + mask2 / (mask2.sum(1, keepdims=True) + EPS) * (1.0 + probs) * ub)
    return np.float32(d.sum())


_BUILT = None


def _build(repeat=1):
    """Build the single-core SPMD Bass module (same program on all 8 cores).

    repeat>1 duplicates the whole per-core computation serially inside one
    NEFF — used only for timing (device time >> launch overhead)."""
    from contextlib import ExitStack

    import concourse.bacc as bacc
    import concourse.bass as bass
    import concourse.mybir as mybir
    import concourse.tile as tile

    f32 = mybir.dt.float32
    u32 = mybir.dt.uint32
    Alu = mybir.AluOpType
    Act = mybir.ActivationFunctionType
    X = mybir.AxisListType.X

    # Bacc (not Bass): its finalize() pipeline runs generate_event_semaphores,
    # which splits multi-semaphore waits — TRN2 instructions encode at most 1.
    nc = bacc.Bacc(None, target_bir_lowering=False, debug=False)
    logits_d = nc.dram_tensor("logits", [NPC, V], f32, kind="ExternalInput")
    # aux rows: 0=rs, 1=re, 2=gt, 3=r1 (1/(c1+eps)), 4=r2 (ub/(V-c1+eps)), 5=V-c1
    aux_d = nc.dram_tensor("aux", [P, 6, TILES], f32, kind="ExternalInput")
    oh_d = nc.dram_tensor("onehot", [P, TILES * NBLK], f32, kind="ExternalInput")
    out_d = nc.dram_tensor("partial", [P, 1], f32, kind="ExternalOutput")

    lg_view = logits_d.rearrange("(t p) v -> t p v", p=P)

    with tile.TileContext(nc) as tc, ExitStack() as ctx:
        singles = ctx.enter_context(tc.tile_pool(name="singles", bufs=1))
        work = ctx.enter_context(tc.tile_pool(name="work", bufs=3))
        m8p = ctx.enter_context(tc.tile_pool(name="m8", bufs=4))
        stage = ctx.enter_context(tc.tile_pool(name="stage", bufs=2))
        ep = ctx.enter_context(tc.tile_pool(name="ep", bufs=2))

        aux = singles.tile([P, 6, TILES], f32)
        nc.gpsimd.dma_start(out=aux, in_=aux_d[:])
        oh = singles.tile([P, TILES, NBLK], f32)
        nc.gpsimd.dma_start(out=oh, in_=oh_d.rearrange("p (t b) -> p t b", b=NBLK))

        pools = {"work": work, "m8": m8p, "stage": stage, "ep": ep}
        for _rep in range(repeat):
            _loop_body(nc, pools, aux, oh, lg_view, out_d)

    nc.finalize()
    return nc


def _loop_body(nc, pools, aux, oh, lg_view, out_d):
    import concourse.mybir as mybir

    f32 = mybir.dt.float32
    u32 = mybir.dt.uint32
    Alu = mybir.AluOpType
    Act = mybir.ActivationFunctionType
    X = mybir.AxisListType.X

    work = pools["work"]
    m8p = pools["m8"]
    stage = pools["stage"]
    ep = pools["ep"]

    if True:
        blocks = stage.tile([P, TILES, NBLK], f32, tag="blocks")
        s_all = stage.tile([P, TILES], f32, tag="s_all")
        idx_all = stage.tile([P, TILES, 8], u32, tag="idx_all")

        for t in range(TILES):
            lg = work.tile([P, V], f32, tag="lg")
            nc.sync.dma_start(out=lg, in_=lg_view[t])
            ex = work.tile([P, V], f32, tag="ex")
            nc.scalar.activation(
                out=ex, in_=lg, func=Act.Exp, accum_out=s_all[:, t : t + 1]
            )
            nc.vector.tensor_reduce(
                out=blocks[:, t, :],
                in_=ex[:, :].rearrange("p (b k) -> p b k", b=NBLK),
                axis=X,
                op=Alu.add,
            )
            # argmax on exp (monotonic in logits): keeps ACT as the only
            # consumer of lg so its DMA carries <=2 descriptor waits.
            m8 = m8p.tile([P, 8], f32, tag="m8")
            nc.vector.max(out=m8, in_=ex)
            nc.vector.max_index(out=idx_all[:, t, :], in_max=m8, in_values=ex)

        # ---- batched epilogue over [P, TILES] ----
        masked = ep.tile([P, TILES, NBLK], f32)
        nc.vector.tensor_tensor(masked, blocks, oh, Alu.mult)
        s_in = ep.tile([P, TILES], f32)
        nc.vector.tensor_reduce(out=s_in, in_=masked, axis=X, op=Alu.add)

        predf = ep.tile([P, TILES], f32)
        nc.vector.tensor_copy(predf, idx_all[:, :, 0])

        recip = ep.tile([P, TILES], f32)
        nc.vector.reciprocal(recip, s_all)

        rs = aux[:, 0, :]
        re_ = aux[:, 1, :]
        gt = aux[:, 2, :]
        r1 = aux[:, 3, :]
        r2 = aux[:, 4, :]
        vmc = aux[:, 5, :]

        ge = ep.tile([P, TILES], f32)
        nc.vector.tensor_tensor(ge, predf, rs, Alu.is_ge)
        lt = ep.tile([P, TILES], f32)
        nc.vector.tensor_tensor(lt, predf, re_, Alu.is_lt)
        same = ep.tile([P, TILES], f32)
        nc.vector.tensor_tensor(same, ge, lt, Alu.mult)

        dist = ep.tile([P, TILES], f32)
        nc.vector.tensor_tensor(dist, predf, gt, Alu.subtract)
        adist = ep.tile([P, TILES], f32)
        nc.scalar.activation(out=adist, in_=dist, func=Act.Abs)

        # term1 = |pred-gt| * s_in * recip * r1
        t1 = ep.tile([P, TILES], f32)
        nc.vector.tensor_tensor(t1, adist, s_in, Alu.mult)
        nc.vector.tensor_tensor(t1, t1, recip, Alu.mult)
        nc.vector.tensor_tensor(t1, t1, r1, Alu.mult)

        # term2 = r2 * (vmc + (S - s_in) * recip)
        t2 = ep.tile([P, TILES], f32)
        nc.vector.tensor_tensor(t2, s_all, s_in, Alu.subtract)
        nc.vector.tensor_tensor(t2, t2, recip, Alu.mult)
        nc.vector.tensor_tensor(t2, t2, vmc, Alu.add)
        nc.vector.tensor_tensor(t2, t2, r2, Alu.mult)

        # res = t2 + same * (t1 - t2)
        res = ep.tile([P, TILES], f32)
        nc.vector.tensor_tensor(res, t1, t2, Alu.subtract)
        nc.vector.tensor_tensor(res, res, same, Alu.mult)
        nc.vector.tensor_tensor(res, res, t2, Alu.add)

        rowsum = ep.tile([P, 1], f32)
        nc.vector.tensor_reduce(out=rowsum, in_=res, axis=X, op=Alu.add)
        nc.gpsimd.dma_start(out=out_d[:], in_=rowsum)
    return nc


def _get_built():
    global _BUILT
    if _BUILT is None:
        _BUILT = _build()
    return _BUILT


def _make_in_maps(inputs):
    """Build per-core input maps, or None if the hardcoded block structure
    does not hold (then the numpy fallback must be used)."""
    logits = np.ascontiguousarray(np.asarray(inputs["logits"], dtype=np.float32))
    gt = np.asarray(inputs["ground_truths"]).astype(np.int64)
    event_ids = np.asarray(inputs["event_ids"]).astype(np.int64)
    range_start = np.asarray(inputs["range_start"]).astype(np.int64)
    range_end = np.asarray(inputs["range_end"]).astype(np.int64)

    blocks_ok = (
        logits.shape == (N, V)
        and gt.shape == (N,)
        and np.array_equal(event_ids, np.arange(V) // BLK)
        and np.array_equal(range_start, (np.arange(V) // BLK) * BLK)
        and np.array_equal(range_end, (np.arange(V) // BLK) * BLK + BLK)
    )
    if not blocks_ok:
        return None

    ub = float(np.max(range_end - range_start))
    rs = range_start[gt].astype(np.float64)
    re_ = range_end[gt].astype(np.float64)
    c1 = re_ - rs
    vmc = V - c1
    aux_rows = np.stack(
        [
            rs,
            re_,
            gt.astype(np.float64),
            1.0 / (c1 + EPS),
            ub / (vmc + EPS),
            vmc,
        ]
    ).astype(np.float32)  # [6, N]

    blk_idx = (rs / BLK).astype(np.int64)  # gt's block per row
    onehot = np.zeros((N, NBLK), dtype=np.float32)
    onehot[np.arange(N), blk_idx] = 1.0

    in_maps = []
    for c in range(NCORES):
        sl = slice(c * NPC, (c + 1) * NPC)
        # device layout: value for row t*P+p lives at [p, t]
        aux_c = (
            aux_rows[:, sl].reshape(6, TILES, P).transpose(2, 0, 1)
        )  # [P, 6, TILES]
        oh_c = (
            onehot[sl].reshape(TILES, P, NBLK).transpose(1, 0, 2).reshape(P, TILES * NBLK)
        )
        in_maps.append(
            {
                "logits": logits[sl],
                "aux": np.ascontiguousarray(aux_c),
                "onehot": np.ascontiguousarray(oh_c),
            }
        )
    return in_maps


def kernel(**inputs):
    in_maps = _make_in_maps(inputs)
    if in_maps is None:
        return _np_loss(
            np.asarray(inputs["logits"], dtype=np.float32),
            np.asarray(inputs["ground_truths"]).astype(np.int64),
            np.asarray(inputs["event_ids"]).astype(np.int64),
            np.asarray(inputs["range_start"]).astype(np.int64),
            np.asarray(inputs["range_end"]).astype(np.int64),
        )

    from concourse.bass_utils import run_bass_kernel_spmd

    nc = _get_built()
    res = run_bass_kernel_spmd(nc, in_maps, list(range(NCORES)))
    total = np.float64(0.0)
    for r in res.results:
        total += r["partial"].astype(np.float64).sum()
    return np.float32(total)


# revision 25
# speedup vs baseline: 1.1044x; 1.1044x over previous
"""Trainium2 Bass kernel for nn_CustomLoss_60885456388844.

Masked-distance custom loss over logits [65536, 1024]:
  probs = exp(logits) / (sum_exp + eps)            per row
  pred  = argmax(logits)                           per row
  same_event = event_ids[pred] == event_ids[gt]
  term1 (same_event): |pred-gt| * sum_{gt range} probs / (range_len + eps)
  term2 (else):       ub * sum_{outside gt range} (1+probs) / (V-range_len + eps)
  loss = sum over rows

Sharding: pure data-parallel on the row axis across 8 NeuronCores
(8192 rows each). Each core reduces its rows to a [128] partial vector;
the host sums the 8x128 partials into the scalar loss.

Per-core device plan (64 tiles of [128 rows x 1024 vocab]):
  DMA   : logits tile -> SBUF
  ScalarE: exp(tile), accum_out -> per-row sum S (free)
  VectorE: 16 per-row block sums of exp (one tensor_reduce)
           max + max_index -> per-row argmax
  Epilogue (batched [128, 64] ops): one-hot block select for the
  gt-range exp sum, then the closed-form per-row loss and a final
  free-axis reduction to [128, 1].
"""

import numpy as np

N = 65536
V = 1024
NCORES = 8
NPC = N // NCORES          # rows per core
P = 128                    # SBUF partitions
TILES = NPC // P           # row tiles per core
NBLK = 16                  # token-range blocks per row
BLK = V // NBLK            # tokens per block
EPS = 1e-10

# Block-sum offload: tiles with (t % 16) < GP_FRAC compute their per-block
# exp sums on the (otherwise idle) GPSIMD engine via a pairwise fold tree;
# the rest use one DVE tensor_reduce. Balances DVE (max+max_index bound)
# against Pool at ~2.77x per-element cost.
GP_FRAC = 14


def _np_loss(logits, gt, event_ids, range_start, range_end):
    """Exact-semantics numpy fallback (only used if the vocab tables do not
    have the contiguous 64-token block structure this kernel hardcodes)."""
    lg = logits.astype(np.float64)
    exp = np.exp(lg)
    sum_exp = exp.sum(axis=1, keepdims=True) + EPS
    probs = exp / sum_exp
    pred = lg.argmax(axis=1)
    ub = float(np.max(range_end - range_start))
    same = event_ids[pred] == event_ids[gt]
    rs = range_start[gt][:, None]
    re_ = range_end[gt][:, None]
    col = np.arange(V)[None, :]
    in_range = (col >= rs) & (col < re_)
    mask1 = (same[:, None] & in_range).astype(np.float64)
    mask2 = np.where(same[:, None], 0.0, np.where(in_range, 0.0, 1.0))
    tok_dist = np.abs(pred - gt).astype(np.float64)[:, None]
    d = (tok_dist * probs * mask1 / (mask1.sum(1, keepdims=True) + EPS)
         + mask2 / (mask2.sum(1, keepdims=True) + EPS) * (1.0 + probs) * ub)
    return np.float32(d.sum())


_BUILT = None


def _build(repeat=1):
    """Build the single-core SPMD Bass module (same program on all 8 cores).

    repeat>1 duplicates the whole per-core computation serially inside one
    NEFF — used only for timing (device time >> launch overhead)."""
    from contextlib import ExitStack

    import concourse.bacc as bacc
    import concourse.bass as bass
    import concourse.mybir as mybir
    import concourse.tile as tile

    f32 = mybir.dt.float32
    u32 = mybir.dt.uint32
    Alu = mybir.AluOpType
    Act = mybir.ActivationFunctionType
    X = mybir.AxisListType.X

    # Bacc (not Bass): its finalize() pipeline runs generate_event_semaphores,
    # which splits multi-semaphore waits — TRN2 instructions encode at most 1.
    nc = bacc.Bacc(None, target_bir_lowering=False, debug=False)
    logits_d = nc.dram_tensor("logits", [NPC, V], f32, kind="ExternalInput")
    # aux rows: 0=rs, 1=re, 2=gt, 3=r1 (1/(c1+eps)), 4=r2 (ub/(V-c1+eps)), 5=V-c1
    aux_d = nc.dram_tensor("aux", [P, 6, TILES], f32, kind="ExternalInput")
    oh_d = nc.dram_tensor("onehot", [P, TILES * NBLK], f32, kind="ExternalInput")
    out_d = nc.dram_tensor("partial", [P, 1], f32, kind="ExternalOutput")

    lg_view = logits_d.rearrange("(t p) v -> t p v", p=P)

    with tile.TileContext(nc) as tc, ExitStack() as ctx:
        singles = ctx.enter_context(tc.tile_pool(name="singles", bufs=1))
        work = ctx.enter_context(tc.tile_pool(name="work", bufs=3))
        m8p = ctx.enter_context(tc.tile_pool(name="m8", bufs=4))
        stage = ctx.enter_context(tc.tile_pool(name="stage", bufs=2))
        ep = ctx.enter_context(tc.tile_pool(name="ep", bufs=2))
        fold = ctx.enter_context(tc.tile_pool(name="fold", bufs=3))

        aux = singles.tile([P, 6, TILES], f32)
        nc.gpsimd.dma_start(out=aux, in_=aux_d[:])
        oh = singles.tile([P, TILES, NBLK], f32)
        nc.gpsimd.dma_start(out=oh, in_=oh_d.rearrange("p (t b) -> p t b", b=NBLK))

        pools = {"work": work, "m8": m8p, "stage": stage, "ep": ep, "fold": fold}
        for _rep in range(repeat):
            _loop_body(nc, pools, aux, oh, lg_view, out_d)

    nc.finalize()
    return nc


def _loop_body(nc, pools, aux, oh, lg_view, out_d):
    import concourse.mybir as mybir

    f32 = mybir.dt.float32
    u32 = mybir.dt.uint32
    Alu = mybir.AluOpType
    Act = mybir.ActivationFunctionType
    X = mybir.AxisListType.X

    work = pools["work"]
    m8p = pools["m8"]
    stage = pools["stage"]
    ep = pools["ep"]
    fold = pools["fold"]

    if True:
        blocks = stage.tile([P, TILES, NBLK], f32, tag="blocks")
        s_all = stage.tile([P, TILES], f32, tag="s_all")
        idx_all = stage.tile([P, TILES, 8], u32, tag="idx_all")

        for t in range(TILES):
            lg = work.tile([P, V], f32, tag="lg")
            nc.sync.dma_start(out=lg, in_=lg_view[t])
            ex = work.tile([P, V], f32, tag="ex")
            nc.scalar.activation(
                out=ex, in_=lg, func=Act.Exp, accum_out=s_all[:, t : t + 1]
            )
            ex3 = ex[:, :].rearrange("p (b k) -> p b k", b=NBLK)
            if t % 16 < GP_FRAC:
                # per-block sums via pairwise fold tree on GPSIMD
                f1 = fold.tile([P, NBLK, 32], f32, tag="f1")
                nc.gpsimd.tensor_tensor(f1, ex3[:, :, 0:32], ex3[:, :, 32:64], Alu.add)
                f2 = fold.tile([P, NBLK, 16], f32, tag="f2")
                nc.gpsimd.tensor_tensor(f2, f1[:, :, 0:16], f1[:, :, 16:32], Alu.add)
                f3 = fold.tile([P, NBLK, 8], f32, tag="f3")
                nc.gpsimd.tensor_tensor(f3, f2[:, :, 0:8], f2[:, :, 8:16], Alu.add)
                f4 = fold.tile([P, NBLK, 4], f32, tag="f4")
                nc.gpsimd.tensor_tensor(f4, f3[:, :, 0:4], f3[:, :, 4:8], Alu.add)
                f5 = fold.tile([P, NBLK, 2], f32, tag="f5")
                nc.gpsimd.tensor_tensor(f5, f4[:, :, 0:2], f4[:, :, 2:4], Alu.add)
                nc.gpsimd.tensor_tensor(
                    blocks[:, t, :], f5[:, :, 0], f5[:, :, 1], Alu.add
                )
            else:
                nc.vector.tensor_reduce(
                    out=blocks[:, t, :], in_=ex3, axis=X, op=Alu.add
                )
            # argmax on exp (monotonic in logits)
            m8 = m8p.tile([P, 8], f32, tag="m8")
            nc.vector.max(out=m8, in_=ex)
            nc.vector.max_index(out=idx_all[:, t, :], in_max=m8, in_values=ex)

        # ---- batched epilogue over [P, TILES] ----
        masked = ep.tile([P, TILES, NBLK], f32)
        nc.vector.tensor_tensor(masked, blocks, oh, Alu.mult)
        s_in = ep.tile([P, TILES], f32)
        nc.vector.tensor_reduce(out=s_in, in_=masked, axis=X, op=Alu.add)

        predf = ep.tile([P, TILES], f32)
        nc.vector.tensor_copy(predf, idx_all[:, :, 0])

        recip = ep.tile([P, TILES], f32)
        nc.vector.reciprocal(recip, s_all)

        rs = aux[:, 0, :]
        re_ = aux[:, 1, :]
        gt = aux[:, 2, :]
        r1 = aux[:, 3, :]
        r2 = aux[:, 4, :]
        vmc = aux[:, 5, :]

        ge = ep.tile([P, TILES], f32)
        nc.vector.tensor_tensor(ge, predf, rs, Alu.is_ge)
        lt = ep.tile([P, TILES], f32)
        nc.vector.tensor_tensor(lt, predf, re_, Alu.is_lt)
        same = ep.tile([P, TILES], f32)
        nc.vector.tensor_tensor(same, ge, lt, Alu.mult)

        dist = ep.tile([P, TILES], f32)
        nc.vector.tensor_tensor(dist, predf, gt, Alu.subtract)
        adist = ep.tile([P, TILES], f32)
        nc.scalar.activation(out=adist, in_=dist, func=Act.Abs)

        # term1 = |pred-gt| * s_in * recip * r1
        t1 = ep.tile([P, TILES], f32)
        nc.vector.tensor_tensor(t1, adist, s_in, Alu.mult)
        nc.vector.tensor_tensor(t1, t1, recip, Alu.mult)
        nc.vector.tensor_tensor(t1, t1, r1, Alu.mult)

        # term2 = r2 * (vmc + (S - s_in) * recip)
        t2 = ep.tile([P, TILES], f32)
        nc.vector.tensor_tensor(t2, s_all, s_in, Alu.subtract)
        nc.vector.tensor_tensor(t2, t2, recip, Alu.mult)
        nc.vector.tensor_tensor(t2, t2, vmc, Alu.add)
        nc.vector.tensor_tensor(t2, t2, r2, Alu.mult)

        # res = t2 + same * (t1 - t2)
        res = ep.tile([P, TILES], f32)
        nc.vector.tensor_tensor(res, t1, t2, Alu.subtract)
        nc.vector.tensor_tensor(res, res, same, Alu.mult)
        nc.vector.tensor_tensor(res, res, t2, Alu.add)

        rowsum = ep.tile([P, 1], f32)
        nc.vector.tensor_reduce(out=rowsum, in_=res, axis=X, op=Alu.add)
        nc.gpsimd.dma_start(out=out_d[:], in_=rowsum)
    return nc


def _get_built():
    global _BUILT
    if _BUILT is None:
        _BUILT = _build()
    return _BUILT


def _make_in_maps(inputs):
    """Build per-core input maps, or None if the hardcoded block structure
    does not hold (then the numpy fallback must be used)."""
    logits = np.ascontiguousarray(np.asarray(inputs["logits"], dtype=np.float32))
    gt = np.asarray(inputs["ground_truths"]).astype(np.int64)
    event_ids = np.asarray(inputs["event_ids"]).astype(np.int64)
    range_start = np.asarray(inputs["range_start"]).astype(np.int64)
    range_end = np.asarray(inputs["range_end"]).astype(np.int64)

    blocks_ok = (
        logits.shape == (N, V)
        and gt.shape == (N,)
        and np.array_equal(event_ids, np.arange(V) // BLK)
        and np.array_equal(range_start, (np.arange(V) // BLK) * BLK)
        and np.array_equal(range_end, (np.arange(V) // BLK) * BLK + BLK)
    )
    if not blocks_ok:
        return None

    ub = float(np.max(range_end - range_start))
    rs = range_start[gt].astype(np.float64)
    re_ = range_end[gt].astype(np.float64)
    c1 = re_ - rs
    vmc = V - c1
    aux_rows = np.stack(
        [
            rs,
            re_,
            gt.astype(np.float64),
            1.0 / (c1 + EPS),
            ub / (vmc + EPS),
            vmc,
        ]
    ).astype(np.float32)  # [6, N]

    blk_idx = (rs / BLK).astype(np.int64)  # gt's block per row
    onehot = np.zeros((N, NBLK), dtype=np.float32)
    onehot[np.arange(N), blk_idx] = 1.0

    in_maps = []
    for c in range(NCORES):
        sl = slice(c * NPC, (c + 1) * NPC)
        # device layout: value for row t*P+p lives at [p, t]
        aux_c = (
            aux_rows[:, sl].reshape(6, TILES, P).transpose(2, 0, 1)
        )  # [P, 6, TILES]
        oh_c = (
            onehot[sl].reshape(TILES, P, NBLK).transpose(1, 0, 2).reshape(P, TILES * NBLK)
        )
        in_maps.append(
            {
                "logits": logits[sl],
                "aux": np.ascontiguousarray(aux_c),
                "onehot": np.ascontiguousarray(oh_c),
            }
        )
    return in_maps


def kernel(**inputs):
    in_maps = _make_in_maps(inputs)
    if in_maps is None:
        return _np_loss(
            np.asarray(inputs["logits"], dtype=np.float32),
            np.asarray(inputs["ground_truths"]).astype(np.int64),
            np.asarray(inputs["event_ids"]).astype(np.int64),
            np.asarray(inputs["range_start"]).astype(np.int64),
            np.asarray(inputs["range_end"]).astype(np.int64),
        )

    from concourse.bass_utils import run_bass_kernel_spmd

    nc = _get_built()
    res = run_bass_kernel_spmd(nc, in_maps, list(range(NCORES)))
    total = np.float64(0.0)
    for r in res.results:
        total += r["partial"].astype(np.float64).sum()
    return np.float32(total)
